# revision 1
# baseline (speedup 1.0000x reference)
"""Trainium2 Bass kernel for nn_Actor_att (dense_transformer, 8 cores DP).

Layout: feature-major [features(partitions), batch(free)].  Per 1024-sample
macro-tile: PE transposes x to xT, small block-diagonal matmuls run all three
input branches stacked, attention scores/softmax/pooling via PE reductions +
DVE elementwise, LayerNorm computed on UNNORMALIZED pooled values with the
softmax 1/Z folded exactly into the eps term:
    LN(v/Z) = (v - mean(v)) / sqrt(E[v^2] - mean(v)^2 + eps*Z^2)
rstd is computed as exp(-0.5*ln(var')) (ACT LUT; Rsqrt is blocked as
inaccurate and DVE reciprocal is 8 cycles/elem).

This walrus build accepts at most ONE sync wait per instruction (2 for
InstEventSemaphore); Tile emits more.  split_waits() redistributes excess
waits onto earlier same-engine carrier instructions (seeded nops), which is
sound: a wait may fire earlier than needed, and a carrier placed after the
wait's producer in schedule order can never deadlock.
"""
import math
from collections import defaultdict

import numpy as np

N_CORES = 8
B = 524288
BS = B // N_CORES          # 65536 samples per core
NT_TILE = 512              # samples per macro-tile
N_TILES = BS // NT_TILE
CHUNK = 512                # matmul free-dim chunk (one PSUM bank)
NCH = NT_TILE // CHUNK     # matmul chunks per tile
NTR = NT_TILE // 128       # PE transposes per tile
_PS_BANKS_PER_TILE = max(1, NT_TILE * 4 // 2048)
PS_BUFS = 8 // _PS_BANKS_PER_TILE
GRP = 4
IO_BUFS = 6
EPS = 1e-5

_CACHE = {}


# ---------------------------------------------------------------- constants
def _build_consts(w):
    """Pack all weights/patterns into one [128, CW] f32 matrix."""
    C = {}
    cols = [0]

    def put(name, arr):
        arr = np.asarray(arr, np.float32)
        if arr.ndim == 1:
            arr = arr[:, None]
        C[name] = (cols[0], arr)
        cols[0] += arr.shape[1]

    ident = np.eye(128, dtype=np.float32)
    put("ident", ident)

    cl1a = np.zeros((12, 128), np.float32)
    cl1a[0:2, 0:32] = w["Ws1"]
    for n in range(3):
        cl1a[2 + 2 * n:4 + 2 * n, 32 + 32 * n:64 + 32 * n] = w["Wf1"]
    put("cl1a", cl1a)
    cl1b = np.zeros((12, 64), np.float32)
    for m in range(2):
        cl1b[8 + 2 * m:10 + 2 * m, 32 * m:32 * m + 32] = w["Wo1"]
    put("cl1b", cl1b)
    put("b1", np.concatenate([w["bs1"], np.tile(w["bf1"], 3)]))
    put("b2o", np.tile(w["bo1"], 2))

    cl2 = np.zeros((128, 64), np.float32)
    cl2[0:32, 0:16] = w["Ws2"]
    for n in range(3):
        cl2[32 + 32 * n:64 + 32 * n, 16 + 16 * n:32 + 16 * n] = w["Wf2"]
    put("cl2", cl2)
    cl2o = np.zeros((64, 32), np.float32)
    for m in range(2):
        cl2o[32 * m:32 * m + 32, 16 * m:16 * m + 16] = w["Wo2"]
    put("cl2o", cl2o)
    put("b2", np.concatenate([w["bs2"], np.tile(w["bf2"], 3),
                              np.tile(w["bo2"], 2)]))
    put("b2ob", np.tile(w["bo2"], 2))

    crep = np.zeros((16, 96), np.float32)
    cs = np.zeros((96, 5), np.float32)
    ceb = np.zeros((5, 96), np.float32)
    cp = np.zeros((96, 32), np.float32)
    for n in range(3):
        for d in range(16):
            crep[d, 16 + 16 * n + d] = 1.0
            cs[16 + 16 * n + d, n] = 0.25
            ceb[n, 16 + 16 * n + d] = 1.0
            cp[16 + 16 * n + d, d] = 1.0
    for m in range(2):
        for d in range(16):
            crep[d, 64 + 16 * m + d] = 1.0
            cs[64 + 16 * m + d, 3 + m] = 0.25
            ceb[3 + m, 64 + 16 * m + d] = 1.0
            cp[64 + 16 * m + d, 16 + d] = 1.0
    put("crep", crep)
    put("cs", cs)
    put("ceb", ceb)
    put("cp", cp)
    cz = np.zeros((5, 2), np.float32)
    cz[0:3, 0] = 1.0
    cz[3:5, 1] = 1.0
    put("cz", cz)

    cstm = np.zeros((34, 2), np.float32)
    cste = np.zeros((34, 2), np.float32)
    cstm[0:16, 0] = 1 / 16
    cstm[16:32, 1] = 1 / 16
    cste[0:16, 0] = 1 / 16
    cste[16:32, 1] = 1 / 16
    cste[32, 0] = EPS
    cste[33, 1] = EPS
    put("cstm", cstm)
    put("cste", cste)

    cga = np.zeros((2, 32), np.float32)
    cga[0, 0:16] = w["gf"]
    cga[1, 16:32] = w["go"]
    put("cga", cga)

    put("cm1a", w["Wm1"][16:48, :])
    put("cm1b", w["Wm1"][0:16, :])
    put("bm1", w["bm1"])
    put("cm2", w["Wm2"])
    put("bm2", w["bm2"])
    put("cm3", w["Wm3"])
    put("bm3", w["bm3"])
    put("beta", np.concatenate([w["betaf"], w["betao"]]))

    CW = cols[0]
    cst = np.zeros((128, CW), np.float32)
    for name, (c0, arr) in C.items():
        cst[0:arr.shape[0], c0:c0 + arr.shape[1]] = arr
    slices = {name: (c0, arr.shape) for name, (c0, arr) in C.items()}
    return cst, slices


# ---------------------------------------------------------- wait splitting
def _split_waits(nc):
    import concourse.mybir as mybir

    def cap(inst):
        return 2 if type(inst).__name__ == "InstEventSemaphore" else 1

    order = [i for bb in nc.m.functions[0].blocks for i in bb.instructions]
    cum = defaultdict(int)
    hist = defaultdict(list)
    for p, inst in enumerate(order):
        si = inst.sync_info
        if not si:
            continue
        for u in si.on_update:
            val = u.update_value if u.uses_immediate else 1
            cum[u.id] += val or 1
            hist[u.id].append((p, cum[u.id]))

    def producer_pos(sid, need):
        for p, c in hist[sid]:
            if c >= need:
                return p
        return None

    def free_slots(inst):
        si = inst.sync_info
        return cap(inst) - (len(si.on_wait) if si else 0)

    eng_insts = defaultdict(list)
    for p, inst in enumerate(order):
        eng_insts[str(inst.engine)].append(p)

    failures = []
    for p, inst in enumerate(order):
        si = inst.sync_info
        if not si or len(si.on_wait) <= cap(inst):
            continue
        waits = list(si.on_wait)
        movable_ids = {id(w) for w in waits
                       if w.sync_type == "semaphore"
                       and "ge" in str(w.wait_mode)}
        if len(waits) - len(movable_ids) > cap(inst):
            failures.append((p, inst.name, -1, -1, -1))
            continue
        wp = []
        for w in waits:
            pp = producer_pos(w.id, w.wait_value)
            wp.append(-1 if pp is None else pp)
        n_fixed = len(waits) - len(movable_ids)
        mv = sorted((i for i, w in enumerate(waits) if id(w) in movable_ids),
                    key=lambda i: -wp[i])
        excess = [(waits[i], wp[i]) for i in mv[max(0, cap(inst) - n_fixed):]]
        epos = [q for q in eng_insts[str(inst.engine)] if q < p]
        epos.reverse()
        placed = []
        for w, wprod in excess:
            carrier = None
            for q in epos:
                if free_slots(order[q]) <= 0:
                    continue
                if q <= wprod:
                    break
                carrier = order[q]
                break
            if carrier is None:
                failures.append((p, type(inst).__name__ + ":" + inst.name,
                                 w.id, w.wait_value, wprod))
                continue
            csi = carrier.sync_info
            cw = (list(csi.on_wait) if csi else []) + [w]
            cu = list(csi.on_update) if csi else []
            carrier.sync_info = mybir.SyncInfo(on_wait=cw, on_update=cu)
            placed.append(id(w))
        if placed:
            kept = [w for w in waits if id(w) not in placed]
            inst.sync_info = mybir.SyncInfo(on_wait=kept,
                                            on_update=list(si.on_update))
    if failures:
        raise RuntimeError(f"split_waits failures: {failures[:8]} "
                           f"({len(failures)} total)")
    bad = [i.name for i in order
           if i.sync_info and len(i.sync_info.on_wait) > cap(i)]
    assert not bad, bad


# ----------------------------------------------------------------- builder
def _build_nc(n_tiles, cw, csl):
    import concourse.bass as bass
    import concourse.mybir as mybir
    import concourse.tile as tile
    from concourse.tile import add_dep_helper

    f32 = mybir.dt.float32
    AF = mybir.ActivationFunctionType
    ALU = mybir.AluOpType

    f32r = mybir.dt.float32r
    nc = bass.Bass()
    x_d = nc.declare_dram_parameter("x", [n_tiles * NT_TILE * 12], f32r,
                                    isOutput=False)
    c_d = nc.declare_dram_parameter("cst", [128, cw], f32r, isOutput=False)
    o_d = nc.declare_dram_parameter("out", [n_tiles * NT_TILE * 2], f32,
                                    isOutput=True)

    def sop(eng, handle, nops=1, aft=()):
        """pin `nops` carrier nops after the op's producers (split_waits
        moves excess sem waits onto them)."""
        afters = [a for a in aft if a is not None]
        for _ in range(nops):
            n = eng.nop()
            add_dep_helper(handle.ins, n.ins, sync=False)
            for a in afters:
                add_dep_helper(n.ins, a.ins, sync=False)
        return handle

    final_ops = []
    recent = {"m": None}

    with tile.TileContext(nc) as tc:
        with tc.tile_pool(name="con", bufs=1) as con, \
             tc.tile_pool(name="io", bufs=IO_BUFS) as iop, \
             tc.tile_pool(name="wk1", bufs=1) as wk1, \
             tc.tile_pool(name="wk2", bufs=2) as wk2, \
             tc.tile_pool(name="ps", bufs=PS_BUFS, space="PSUM") as ps:

            cst = con.tile([128, cw], f32r)
            dcst = nc.sync.dma_start(out=cst[:, :], in_=c_d[:, :])

            def r32(ap):
                return ap.bitcast(mybir.dt.float32r)

            def cs_(name, r0=0):
                c0, shp = csl[name]
                return r32(cst[r0:shp[0], c0:c0 + shp[1]])

            def cvec(name, p):  # [p,1] bias vector (plain f32 view)
                c0, shp = csl[name]
                return cst[0:p, c0:c0 + 1].bitcast(f32)

            def tile_stages(t, sfx):
                xbm = iop.tile([128, NT_TILE * 12 // 128], f32r,
                               tag="xbm" + sfx)
                din = sop(nc.sync, nc.sync.dma_start(
                    out=xbm[:, :],
                    in_=x_d[t * NT_TILE * 12:(t + 1) * NT_TILE * 12].rearrange(
                        "(p c) -> p c", p=128)), nops=2,
                    aft=(recent["m"] or dcst,))
                yield

                pxT = ps.tile([12, NT_TILE], f32, tag="pp")
                m = None
                for c in range(NTR):
                    m = sop(nc.tensor, nc.tensor.transpose(
                        r32(pxT[:, c * 128:(c + 1) * 128]),
                        r32(xbm[:, 12 * c:12 * (c + 1)]),
                        r32(cst[:, 0:128])), aft=(din,))
                recent["m"] = m
                yield
                xT = wk1.tile([12, NT_TILE], f32r, tag="xT" + sfx)
                a_xT = sop(nc.scalar, nc.scalar.activation(
                    xT[:, :], pxT[:, :], AF.Copy), aft=(m,))
                yield

                ph1 = ps.tile([128, NT_TILE], f32, tag="pp")
                ph2 = ps.tile([64, NT_TILE], f32, tag="pp")
                m1 = m2 = None
                for c in range(NCH):
                    sl = slice(c * CHUNK, (c + 1) * CHUNK)
                    m1 = sop(nc.tensor, nc.tensor.matmul(
                        ph1[:, sl], cs_("cl1a"), r32(xT[:, sl]),
                        start=True, stop=True), aft=(a_xT,))
                    m2 = sop(nc.tensor, nc.tensor.matmul(
                        ph2[:, sl], cs_("cl1b"), r32(xT[:, sl]),
                        start=True, stop=True), aft=(a_xT,))
                yield
                h1 = wk1.tile([128, NT_TILE], f32r, tag="h1" + sfx)
                a_h1 = sop(nc.scalar, nc.scalar.activation(
                    h1[:, :], ph1[:, :], AF.Relu, bias=cvec("b1", 128)),
                    aft=(m1,))
                h2 = wk1.tile([64, NT_TILE], f32r, tag="h2" + sfx)
                v_h2 = sop(nc.vector, nc.vector.tensor_scalar(
                    h2[:, :], ph2[:, :], cvec("b2o", 64), 0.0,
                    ALU.add, ALU.max), aft=(m2,))
                yield

                pkv = ps.tile([64, NT_TILE], f32, tag="pp")
                pkvo = ps.tile([32, NT_TILE], f32, tag="pp")
                mkv = mkvo = None
                for c in range(NCH):
                    sl = slice(c * CHUNK, (c + 1) * CHUNK)
                    mkv = sop(nc.tensor, nc.tensor.matmul(
                        pkv[:, sl], cs_("cl2"), r32(h1[:, sl]),
                        start=True, stop=True), aft=(a_h1,))
                    mkvo = sop(nc.tensor, nc.tensor.matmul(
                        pkvo[:, sl], cs_("cl2o"), r32(h2[:, sl]),
                        start=True, stop=True), aft=(v_h2,))
                yield
                SK = wk2.tile([96, NT_TILE], f32r, tag="SK" + sfx)
                a_SK = sop(nc.scalar, nc.scalar.activation(
                    SK[0:64, :], pkv[:, :], AF.Relu, bias=cvec("b2", 64)),
                    aft=(mkv,))
                v_SKo = sop(nc.vector, nc.vector.tensor_scalar(
                    SK[64:96, :], pkvo[:, :], cvec("b2ob", 32), 0.0,
                    ALU.add, ALU.max), aft=(mkvo,))
                yield

                pq = ps.tile([96, NT_TILE], f32, tag="pp")
                mq = None
                for c in range(NCH):
                    sl = slice(c * CHUNK, (c + 1) * CHUNK)
                    mq = sop(nc.tensor, nc.tensor.matmul(
                        pq[:, sl], cs_("crep", 0), r32(SK[0:16, sl]),
                        start=True, stop=True), aft=(a_SK,))
                yield
                SW = wk1.tile([96, NT_TILE], f32r, tag="SW" + sfx)
                v_SW = sop(nc.vector, nc.vector.tensor_mul(
                    SW[:, :], pq[:, :], SK[:, :]), nops=2,
                    aft=(mq, a_SK, v_SKo))
                yield

                ps_ = ps.tile([5, NT_TILE], f32, tag="pp")
                msc = None
                for c in range(NCH):
                    sl = slice(c * CHUNK, (c + 1) * CHUNK)
                    msc = sop(nc.tensor, nc.tensor.matmul(
                        ps_[:, sl], cs_("cs"), r32(SW[:, sl]),
                        start=True, stop=True), aft=(v_SW,))
                yield
                E = wk2.tile([5, NT_TILE], f32r, tag="E" + sfx)
                a_E = sop(nc.scalar, nc.scalar.activation(
                    E[:, :], ps_[:, :], AF.Exp), aft=(msc,))
                yield

                peb = ps.tile([96, NT_TILE], f32, tag="pp")
                meb = None
                for c in range(NCH):
                    sl = slice(c * CHUNK, (c + 1) * CHUNK)
                    meb = sop(nc.tensor, nc.tensor.matmul(
                        peb[:, sl], cs_("ceb"), r32(E[:, sl]),
                        start=True, stop=True), aft=(a_E,))
                yield
                WV = wk1.tile([96, NT_TILE], f32r, tag="WV" + sfx)
                v_WV = sop(nc.vector, nc.vector.tensor_mul(
                    WV[:, :], peb[:, :], SK[:, :]), nops=2, aft=(meb, a_SK))
                yield

                ppz = ps.tile([32, NT_TILE], f32, tag="pp")
                pz = ps.tile([2, NT_TILE], f32, tag="pp")
                mpz = mz = None
                for c in range(NCH):
                    sl = slice(c * CHUNK, (c + 1) * CHUNK)
                    mpz = sop(nc.tensor, nc.tensor.matmul(
                        ppz[:, sl], cs_("cp"), r32(WV[:, sl]),
                        start=True, stop=True), aft=(v_WV,))
                    mz = sop(nc.tensor, nc.tensor.matmul(
                        pz[:, sl], cs_("cz"), r32(E[:, sl]),
                        start=True, stop=True), aft=(a_E,))
                yield
                V = wk2.tile([34, NT_TILE], f32r, tag="V" + sfx)
                a_V = sop(nc.scalar, nc.scalar.activation(
                    V[0:32, :], ppz[:, :], AF.Copy), aft=(mpz,))
                v_Vz = sop(nc.vector, nc.vector.tensor_copy(
                    V[32:34, :], pz[:, :]), aft=(mz,))
                yield
                SQ = wk1.tile([34, NT_TILE], f32r, tag="SQ" + sfx)
                v_SQ = sop(nc.vector, nc.vector.tensor_mul(
                    SQ[:, :], V[:, :], V[:, :]), nops=2, aft=(a_V, v_Vz))
                yield

                pst1 = ps.tile([2, NT_TILE], f32, tag="pp")
                pst2 = ps.tile([2, NT_TILE], f32, tag="pp")
                ms1 = ms2 = None
                for c in range(NCH):
                    sl = slice(c * CHUNK, (c + 1) * CHUNK)
                    ms1 = sop(nc.tensor, nc.tensor.matmul(
                        pst1[:, sl], cs_("cstm"), r32(V[:, sl]),
                        start=True, stop=True), aft=(a_V, v_Vz))
                    ms2 = sop(nc.tensor, nc.tensor.matmul(
                        pst2[:, sl], cs_("cste"), r32(SQ[:, sl]),
                        start=True, stop=True), aft=(v_SQ,))
                yield
                STM = wk1.tile([2, NT_TILE], f32, tag="STM" + sfx)
                v_STM = sop(nc.scalar, nc.scalar.activation(
                    STM[:, :], pst1[:, :], AF.Copy), aft=(ms1,))
                MSQ = wk1.tile([2, NT_TILE], f32, tag="xT" + sfx)
                v_MSQ = sop(nc.gpsimd, nc.gpsimd.tensor_mul(
                    MSQ[:, :], STM[:, :], STM[:, :]), nops=3, aft=(v_STM,))
                VAR = wk1.tile([2, NT_TILE], f32, tag="VAR" + sfx)
                v_VAR = sop(nc.vector, nc.vector.tensor_sub(
                    VAR[:, :], pst2[:, :], MSQ[:, :]), nops=2,
                    aft=(ms2, v_MSQ))
                yield
                LNV = wk1.tile([2, NT_TILE], f32, tag="h2" + sfx)
                a_LNV = sop(nc.scalar, nc.scalar.activation(
                    LNV[:, :], VAR[:, :], AF.Ln), aft=(v_VAR,))
                RS = wk1.tile([2, NT_TILE], f32r, tag="RS" + sfx)
                a_RS = sop(nc.scalar, nc.scalar.activation(
                    RS[:, :], LNV[:, :], AF.Exp, scale=-0.5), aft=(a_LNV,))
                MA = wk1.tile([2, NT_TILE], f32r, tag="MA" + sfx)
                v_MA = sop(nc.gpsimd, nc.gpsimd.tensor_mul(
                    MA[:, :], STM[:, :], RS[:, :]), nops=3, aft=(a_RS,))
                yield

                pgb1 = ps.tile([32, NT_TILE], f32, tag="pp")
                pgb2 = ps.tile([32, NT_TILE], f32, tag="pp")
                mg1 = mg2 = None
                for c in range(NCH):
                    sl = slice(c * CHUNK, (c + 1) * CHUNK)
                    mg1 = sop(nc.tensor, nc.tensor.matmul(
                        pgb1[:, sl], cs_("cga"), r32(RS[:, sl]),
                        start=True, stop=True), aft=(a_RS,))
                    mg2 = sop(nc.tensor, nc.tensor.matmul(
                        pgb2[:, sl], cs_("cga"), r32(MA[:, sl]),
                        start=True, stop=True), aft=(v_MA,))
                yield
                T1 = wk1.tile([32, NT_TILE], f32, tag="T1" + sfx)
                v_T1 = sop(nc.vector, nc.vector.tensor_mul(
                    T1[:, :], pgb1[:, :], V[0:32, :]), nops=2,
                    aft=(mg1, a_V))
                U = wk1.tile([32, NT_TILE], f32, tag="SW" + sfx)
                v_U = sop(nc.vector, nc.vector.tensor_sub(
                    U[:, :], T1[:, :], pgb2[:, :]), aft=(mg2, v_T1))
                LNR = wk1.tile([32, NT_TILE], f32r, tag="LNR" + sfx)
                a_LNR = sop(nc.scalar, nc.scalar.activation(
                    LNR[:, :], U[:, :], AF.Relu, bias=cvec("beta", 32)),
                    aft=(v_U,))
                yield

                pm1 = ps.tile([32, NT_TILE], f32, tag="pp")
                mm1 = None
                for c in range(NCH):
                    sl = slice(c * CHUNK, (c + 1) * CHUNK)
                    sop(nc.tensor, nc.tensor.matmul(
                        pm1[:, sl], cs_("cm1a"), r32(LNR[:, sl]),
                        start=True, stop=False), aft=(a_LNR,))
                    mm1 = sop(nc.tensor, nc.tensor.matmul(
                        pm1[:, sl], cs_("cm1b"), r32(SK[0:16, sl]),
                        start=False, stop=True), aft=(a_SK,))
                yield
                H1 = wk1.tile([32, NT_TILE], f32r, tag="H1" + sfx)
                a_H1 = sop(nc.scalar, nc.scalar.activation(
                    H1[:, :], pm1[:, :], AF.Lrelu, bias=cvec("bm1", 32),
                    alpha=0.01), aft=(mm1,))
                yield
                pm2 = ps.tile([32, NT_TILE], f32, tag="pp")
                mm2_ = None
                for c in range(NCH):
                    sl = slice(c * CHUNK, (c + 1) * CHUNK)
                    mm2_ = sop(nc.tensor, nc.tensor.matmul(
                        pm2[:, sl], cs_("cm2"), r32(H1[:, sl]),
                        start=True, stop=True), aft=(a_H1,))
                yield
                H2 = wk1.tile([32, NT_TILE], f32r, tag="H2" + sfx)
                a_H2 = sop(nc.scalar, nc.scalar.activation(
                    H2[:, :], pm2[:, :], AF.Lrelu, bias=cvec("bm2", 32),
                    alpha=0.01), aft=(mm2_,))
                yield
                po = ps.tile([2, NT_TILE], f32, tag="pp")
                mo = None
                for c in range(NCH):
                    sl = slice(c * CHUNK, (c + 1) * CHUNK)
                    mo = sop(nc.tensor, nc.tensor.matmul(
                        po[:, sl], cs_("cm3"), r32(H2[:, sl]),
                        start=True, stop=True), aft=(a_H2,))
                yield
                OUTF = wk1.tile([2, NT_TILE], f32r, tag="OUTF" + sfx)
                a_OF = sop(nc.scalar, nc.scalar.activation(
                    OUTF[:, :], po[:, :], AF.Identity, bias=cvec("bm3", 2)),
                    aft=(mo,))
                yield

                pob = ps.tile([128, 2 * NTR], f32, tag="pp")
                mt = None
                for c in range(NTR):
                    mt = sop(nc.tensor, nc.tensor.transpose(
                        r32(pob[:, 2 * c:2 * c + 2]),
                        r32(OUTF[:, 128 * c:128 * (c + 1)]),
                        r32(cst[0:2, 0:2])), aft=(a_OF,))
                yield
                obm = iop.tile([128, 2 * NTR], f32, tag="obm" + sfx)
                v_ob = sop(nc.vector, nc.vector.tensor_copy(
                    obm[:, :], pob[:, :]), aft=(mt,))
                dout = sop(nc.gpsimd, nc.gpsimd.dma_start(
                    out=o_d[t * NT_TILE * 2:(t + 1) * NT_TILE * 2].rearrange(
                        "(p c) -> p c", p=128),
                    in_=obm[:, :]), aft=(v_ob,))
                final_ops[:] = [dout, v_ob, a_OF, mt]
                yield

            def drive(gens):
                alive = list(gens)
                while alive:
                    nxt = []
                    for g in alive:
                        try:
                            next(g)
                            nxt.append(g)
                        except StopIteration:
                            pass
                    alive = nxt

            t = 0
            while t + GRP <= n_tiles:
                drive([tile_stages(t + i, chr(65 + i)) for i in range(GRP)])
                t += GRP
            while t < n_tiles:
                drive([tile_stages(t, "A")])
                t += 1

            prev = None
            for _ in range(28):
                n = nc.sync.nop()
                for op in final_ops:
                    add_dep_helper(n.ins, op.ins, sync=False)
                if prev is not None:
                    add_dep_helper(n.ins, prev.ins, sync=False)
                prev = n

    _split_waits(nc)
    return nc


# ------------------------------------------------------------------ runner
def _get_nc(n_tiles, cw, csl):
    key = (n_tiles, cw)
    if key not in _CACHE:
        _CACHE[key] = _build_nc(n_tiles, cw, csl)
    return _CACHE[key]


def _get_executor(nc):
    """Cache the jitted SPMD executable so repeated kernel() calls only pay
    transfer + execute (run_bass_via_pjrt re-traces on every call)."""
    if "fn" in _CACHE:
        return _CACHE["fn"]
    import jax
    import concourse.mybir as mybir
    from jax.sharding import Mesh, PartitionSpec, NamedSharding
    from jax.experimental.shard_map import shard_map
    from concourse import bass2jax
    from concourse.bass2jax import _bass_exec_p, install_neuronx_cc_hook

    install_neuronx_cc_hook()
    partition_name = (nc.partition_id_tensor.name
                      if nc.partition_id_tensor else None)
    in_names, out_names, out_avals, zero_outs = [], [], [], []
    for alloc in nc.m.functions[0].allocations:
        if not isinstance(alloc, mybir.MemoryLocationSet):
            continue
        name = alloc.memorylocations[0].name
        if alloc.kind == "ExternalInput":
            if name != partition_name:
                in_names.append(name)
        elif alloc.kind == "ExternalOutput":
            dt = mybir.dt.np(alloc.dtype)
            out_names.append(name)
            out_avals.append(jax.core.ShapedArray(tuple(alloc.tensor_shape),
                                                  dt))
            zero_outs.append(np.zeros(tuple(alloc.tensor_shape), dt))
    all_names = in_names + out_names + ([partition_name]
                                        if partition_name else [])
    n_params = len(in_names)

    def _body(*args):
        operands = list(args)
        if partition_name is not None:
            operands.append(bass2jax.partition_id_tensor())
        return tuple(_bass_exec_p.bind(
            *operands, out_avals=tuple(out_avals), in_names=tuple(all_names),
            out_names=tuple(out_names), lowering_input_output_aliases=(),
            sim_require_finite=True, sim_require_nnan=True, nc=nc))

    devices = jax.devices()[:N_CORES]
    mesh = Mesh(np.asarray(devices), ("core",))
    nin = n_params + len(zero_outs)
    fn = jax.jit(shard_map(_body, mesh=mesh,
                           in_specs=(PartitionSpec("core"),) * nin,
                           out_specs=(PartitionSpec("core"),) * len(out_names),
                           check_rep=False), keep_unused=True)
    sh = NamedSharding(mesh, PartitionSpec("core"))
    _CACHE["fn"] = (fn, in_names, zero_outs, sh, jax)
    return _CACHE["fn"]


def kernel(**inputs):
    x = np.ascontiguousarray(np.asarray(inputs["x"], np.float32))
    cst, csl = _build_consts(inputs)
    n = x.shape[0]
    bs = n // N_CORES
    n_tiles = bs // NT_TILE
    nc = _get_nc(n_tiles, cst.shape[1], csl)
    fn, in_names, zero_outs, sh, jax = _get_executor(nc)

    per_core = {"x": [x[i * bs:(i + 1) * bs].reshape(-1)
                      for i in range(N_CORES)],
                "cst": [cst] * N_CORES}
    args = []
    for name in in_names:
        args.append(jax.device_put(
            np.concatenate(per_core[name], axis=0), sh))
    for z in zero_outs:
        args.append(jax.device_put(
            np.zeros((N_CORES * z.shape[0], *z.shape[1:]), z.dtype), sh))
    out_arrs = fn(*args)
    out = np.asarray(out_arrs[0]).reshape(N_CORES, bs * 2)
    return out.reshape(n, 2)

